# revision 2
# baseline (speedup 1.0000x reference)
"""MoE (top-2 routing, 16 experts, silu MLP) on 8 Trainium2 NeuronCores.

Mixed-precision expert parallelism:
  - Host: router (top-2 + softmax), dispatch. Each core owns 2 expert slots
    (largest-8 by token count paired with smallest-8). Within each expert's
    batch, tokens are sorted by ascending combine weight; the first F_s
    (~70% of capacity) low-weight tokens take the "fast" path (layer 1 in
    fp8-e4m3 DoubleRow matmuls), the high-weight rest take the "safe" path
    (layer 1 in bf16). Layer 2 is fp8 for all tokens. Error adds in
    quadrature over tokens, so putting fp8 on the low-weight pairs keeps the
    global rel-err ~1.9e-2 while most FLOPs run at the 2x fp8 rate.
  - Device per slot: h = silu(z) written fp8 straight into SBUF (no DRAM
    round trip; Act engine casts with exact RNE), then y*SW2 = h8 @ w2h
    (fp8 DoubleRow), y streamed out bf16.
  - Host: out = x + sum_e cw_e * (y_e/SW2 + b2_e) scatter-added.

fp8 scales (powers of 2): x*16, w1*512 (silu dequant via act scale 1/8192),
w2*256 (dequant folded into host combine).
"""

import sys
import types

import ml_dtypes
import numpy as np

B, D, E, U, TOPK = 16384, 1024, 16, 4096, 2
N_CORES = 8
S = E // N_CORES
P = 128
KQ1 = D // 256    # 4   fp8 double-k tiles, layer 1
KO1 = D // P      # 8   bf16 k tiles, layer 1
M1 = U // P       # 32
KQ2 = U // 256    # 16  fp8 double-k tiles, layer 2
M2 = D // P       # 8
FFRAC = 0.70      # fast-path fraction of each slot capacity

NCHF = 512        # fast-path chunk width (DoubleRow N)
NCHS = 512        # safe-path chunk width
NCH2 = 512        # layer-2 chunk width
SLAB = 512        # x DMA slab width

SX = 16.0
SW1 = 512.0
SW2 = 256.0
SCL1 = 1.0 / (SX * SW1)

E4 = ml_dtypes.float8_e4m3
BF16 = ml_dtypes.bfloat16

LAST_RESULTS = None
TRACE = False
TRACE_CORES = None


def _install_ntff_hook_shim():
    if "antenv.axon_hooks" in sys.modules:
        return
    try:
        import antenv.axon_hooks  # noqa: F401

        return
    except ImportError:
        pass
    try:
        import antenv
    except ImportError:
        return
    mod = types.ModuleType("antenv.axon_hooks")
    mod._hook = None

    def set_axon_ntff_profile_hook(h):
        mod._hook = h

    def get_axon_ntff_profile_hook():
        return mod._hook

    mod.set_axon_ntff_profile_hook = set_axon_ntff_profile_hook
    mod.get_axon_ntff_profile_hook = get_axon_ntff_profile_hook
    sys.modules["antenv.axon_hooks"] = mod
    antenv.axon_hooks = mod
    try:
        from trn_agent_boot.trn_boot import _ntff_profile_via_ctypes

        hook = _ntff_profile_via_ctypes("/opt/axon/libaxon_pjrt.so")
        if hook is not None:
            mod._hook = hook
    except Exception:
        pass


def _chunks(total, step):
    out = []
    c = 0
    while c < total:
        out.append((c, min(step, total - c)))
        c += step
    return out


def q8(v):
    return np.clip(v, -240.0, 240.0).astype(E4)


def _fsplit(cap):
    f = int(round(FFRAC * cap / 256.0)) * 256
    return min(max(f, 256), cap)


_PROGRAM_CACHE = {}


def _build_program(key):
    if key in _PROGRAM_CACHE:
        return _PROGRAM_CACHE[key]
    caps, fs = key

    import concourse.tile as tile
    from concourse import bacc, mybir

    f32 = mybir.dt.float32
    bf16 = mybir.dt.bfloat16
    f8 = mybir.dt.float8e4
    Silu = mybir.ActivationFunctionType.Silu
    DR = mybir.MatmulPerfMode.DoubleRow

    gs = tuple(c - f for c, f in zip(caps, fs))
    CT = sum(caps)
    FT = sum(fs)
    GT = sum(gs)
    CAP = max(caps)
    soff = [0, caps[0]]
    foff = [0, fs[0]]
    goff = [0, gs[0]]

    nc = bacc.Bacc(None, target_bir_lowering=False, debug=False)
    # fast x: [p, kq, i, n], input dim d = kq*256 + i*128 + p
    xT8 = nc.dram_tensor("xT8", [P, KQ1, 2, FT], f8, kind="ExternalInput")
    # safe x: [p, k, n], d = k*128 + p
    xTb = nc.dram_tensor("xTb", [P, KO1, GT], bf16, kind="ExternalInput")
    w1f = nc.dram_tensor("w1f", [S, M1, P, KQ1, 2, P], f8, kind="ExternalInput")
    w1b = nc.dram_tensor("w1b", [S, M1, P, KO1, P], bf16, kind="ExternalInput")
    b1s = nc.dram_tensor("b1s", [S, P, M1], f32, kind="ExternalInput")
    w2f = nc.dram_tensor("w2f", [S, M2, P, KQ2, 2, P], f8, kind="ExternalInput")
    # y*SW2: [p, m2, n], out dim d = m2*128 + p; cols = [fast | safe] per slot
    yT = nc.dram_tensor("yT", [P, M2, CT], bf16, kind="ExternalOutput")

    n_f_slabs = [len(_chunks(f, SLAB)) for f in fs]
    n_s_slabs = [len(_chunks(g, SLAB)) for g in gs]

    with tile.TileContext(nc) as tc:
        with (
            tc.tile_pool(name="bias", bufs=1) as biasp,
            tc.tile_pool(name="hbuf", bufs=1) as hpool,
            tc.tile_pool(name="x8sb", bufs=sum(n_f_slabs) + 1) as x8pool,
            tc.tile_pool(name="xbsb", bufs=sum(n_s_slabs) + 1) as xbpool,
            tc.tile_pool(name="w1fp", bufs=3) as w1fpool,
            tc.tile_pool(name="w1bp", bufs=3) as w1bpool,
            tc.tile_pool(name="w2fp", bufs=3) as w2fpool,
            tc.tile_pool(name="psum", bufs=6, space="PSUM") as psump,
            tc.tile_pool(name="yt", bufs=6) as ypool,
        ):
            b1_sb = biasp.tile([P, S, M1], f32, tag="b1")
            for s in range(S):
                nc.sync.dma_start(b1_sb[:, s, :], b1s[s])

            # PE clock warmup: the engine idles ~10us at startup waiting for
            # x/w DMA, and the first ~3us of execution run at reduced pstate.
            # Burn that window on throwaway matmuls (inputs: the tiny b1 tile)
            # so real matmuls start at full clock.
            warm = psump.tile([P, 16], f32, tag="ps", name="warm")
            for r in range(32):
                nc.tensor.matmul(
                    warm[:16, :16], b1_sb[:, 0, 0:16], b1_sb[:, 0, 0:16],
                    start=True, stop=True,
                )

            # x slabs; slot-0 immediately (fast/safe interleaved so the first
            # safe chunk isn't starved), slot-1 time-gated so the bulk
            # prefetch doesn't starve the w1 stream during slot-0 l1.
            t_l1 = 30_000 + int(caps[0] * 32 * 2.6)
            x8cs = [[], []]
            xbcs = [[], []]

            # First-m weight tiles, hoisted so their DMAs lead the gpsimd and
            # scalar queues at t=0.
            wtf0 = w1fpool.tile([P, KQ1, 2, P], f8, tag="w1f", name="w1f_pre")
            nc.gpsimd.dma_start(wtf0[:], w1f[0, 0])
            wtb0 = w1bpool.tile([P, KO1, P], bf16, tag="w1b", name="w1b_pre")
            nc.scalar.dma_start(wtb0[:], w1b[0, 0])

            def load_x8(s, si, c0, w, engs):
                xc = x8pool.tile([P, KQ1, 2, SLAB], f8, tag="x8", name=f"x8_{s}_{si}")
                for kq in range(KQ1):
                    for i in range(2):
                        engs[(kq * 2 + i) * len(engs) // 8].dma_start(
                            xc[:, kq, i, :w],
                            xT8[:, kq, i, foff[s] + c0 : foff[s] + c0 + w],
                        )
                x8cs[s].append(xc)

            def load_xb(s, si, c0, w, engs):
                xc = xbpool.tile([P, KO1, SLAB], bf16, tag="xb", name=f"xb_{s}_{si}")
                for k in range(KO1):
                    engs[k * len(engs) // 8].dma_start(
                        xc[:, k, :w],
                        xTb[:, k, goff[s] + c0 : goff[s] + c0 + w],
                    )
                xbcs[s].append(xc)

            fch = {s: _chunks(fs[s], SLAB) for s in range(S)}
            gch = {s: _chunks(gs[s], SLAB) for s in range(S)}
            # slot-0 startup: spread slabs over the three DMA queues so the
            # first m-sweep isn't serialized behind one ~90 GB/s queue.
            x8q = [[nc.sync], [nc.scalar], [nc.gpsimd], [nc.sync], [nc.scalar]]
            for si, (c0, w) in enumerate(fch[0]):
                load_x8(0, si, c0, w, x8q[si % len(x8q)])
            for si, (c0, w) in enumerate(gch[0]):
                load_xb(0, si, c0, w, [nc.sync, nc.scalar] if si == 0 else [nc.sync])
            for si, (c0, w) in enumerate(fch[1]):
                with tc.tile_wait_until((0.20 + 0.07 * si) * t_l1 / 1e6):
                    load_x8(1, si, c0, w, [nc.sync])
            for si, (c0, w) in enumerate(gch[1]):
                with tc.tile_wait_until((0.55 + 0.10 * si) * t_l1 / 1e6):
                    load_xb(1, si, c0, w, [nc.sync])

            for s in range(S):
                # ---- layer 1 ----
                h_sb = hpool.tile([P, M1, CAP], f8, tag="h", name=f"h{s}")
                for m in range(M1):
                    if s == 0 and m == 0:
                        wtf, wtb = wtf0, wtb0
                    else:
                        wtf = w1fpool.tile([P, KQ1, 2, P], f8, tag="w1f", name=f"w1f_{s}_{m}")
                        nc.gpsimd.dma_start(wtf[:], w1f[s, m])
                        wtb = w1bpool.tile([P, KO1, P], bf16, tag="w1b", name=f"w1b_{s}_{m}")
                        # first few on the (empty at startup) Act queue; the
                        # rest on sync behind the x-slab stream.
                        (nc.scalar if s == 0 and m < 3 else nc.sync).dma_start(
                            wtb[:], w1b[s, m]
                        )
                    # fast fp8 chunks -> h[:, m, 0:F]
                    for c0, w in _chunks(fs[s], NCHF):
                        slab, off = divmod(c0, SLAB)
                        xc = x8cs[s][slab]
                        ps = psump.tile([P, NCHF], f32, tag="ps", name=f"psf_{s}_{m}_{c0}")
                        for kq in range(KQ1):
                            nc.tensor.matmul(
                                ps[:, :w],
                                wtf[:, kq],
                                xc[:, kq, :, off : off + w],
                                start=(kq == 0),
                                stop=(kq == KQ1 - 1),
                                perf_mode=DR,
                            )
                        nc.scalar.activation(
                            h_sb[:, m, c0 : c0 + w], ps[:, :w], Silu,
                            bias=b1_sb[:, s, m : m + 1], scale=SCL1,
                        )
                    # safe bf16 chunks -> h[:, m, F:C]
                    for c0, w in _chunks(gs[s], NCHS):
                        slab, off = divmod(c0, SLAB)
                        xc = xbcs[s][slab]
                        ps = psump.tile([P, NCHS], f32, tag="ps", name=f"pss_{s}_{m}_{c0}")
                        for k in range(KO1):
                            nc.tensor.matmul(
                                ps[:, :w],
                                wtb[:, k],
                                xc[:, k, off : off + w],
                                start=(k == 0),
                                stop=(k == KO1 - 1),
                            )
                        nc.scalar.activation(
                            h_sb[:, m, fs[s] + c0 : fs[s] + c0 + w], ps[:, :w], Silu,
                            bias=b1_sb[:, s, m : m + 1],
                        )

                # ---- layer 2: y*SW2 = h8 @ w2h (fp8 DoubleRow) ----
                for m2 in range(M2):
                    wt2 = w2fpool.tile([P, KQ2, 2, P], f8, tag="w2f", name=f"w2f_{s}_{m2}")
                    nc.scalar.dma_start(wt2[:], w2f[s, m2])
                    for c0, w in _chunks(caps[s], NCH2):
                        ps = psump.tile([P, NCH2], f32, tag="ps", name=f"ps2_{s}_{m2}_{c0}")
                        for kq in range(KQ2):
                            nc.tensor.matmul(
                                ps[:, :w],
                                wt2[:, kq],
                                h_sb[:, 2 * kq : 2 * kq + 2, c0 : c0 + w],
                                start=(kq == 0),
                                stop=(kq == KQ2 - 1),
                                perf_mode=DR,
                            )
                        yt = ypool.tile([P, NCH2], bf16, tag="yt", name=f"yt_{s}_{m2}_{c0}")
                        nc.vector.tensor_copy(yt[:, :w], ps[:, :w])
                        nc.gpsimd.dma_start(
                            yT[:, m2, soff[s] + c0 : soff[s] + c0 + w], yt[:, :w]
                        )

    nc.compile()
    _PROGRAM_CACHE[key] = nc
    return nc


def _route(x, w_router, b_router):
    logits = x @ w_router + b_router
    idx2 = np.argpartition(-logits, TOPK, axis=1)[:, :TOPK]
    vals = np.take_along_axis(logits, idx2, axis=1)
    order = np.argsort(-vals, axis=1)
    topk_i = np.take_along_axis(idx2, order, axis=1)
    topk_v = np.take_along_axis(vals, order, axis=1)
    topk_v = topk_v - topk_v.max(axis=1, keepdims=True)
    ew = np.exp(topk_v)
    cw = ew / ew.sum(axis=1, keepdims=True)

    eids = topk_i.ravel()
    toks = np.repeat(np.arange(B, dtype=np.int64), TOPK)
    wts = cw.ravel().astype(np.float32)
    perm = np.argsort(eids, kind="stable")
    toks_s, wts_s = toks[perm], wts[perm]
    counts = np.bincount(eids, minlength=E)
    offs = np.concatenate([[0], np.cumsum(counts)])

    rank = np.argsort(-counts, kind="stable")
    slot_expert = [[int(rank[s * N_CORES + c]) for s in range(S)] for c in range(N_CORES)]
    caps = tuple(
        max(512, int(max(counts[rank[s * N_CORES + c]] for c in range(N_CORES))))
        for s in range(S)
    )
    return toks_s, wts_s, offs, slot_expert, caps


def kernel(x, w_router, b_router, w1, b1, w2, b2):
    _install_ntff_hook_shim()
    from concourse.bass_utils import run_bass_kernel_spmd

    x = np.asarray(x, dtype=np.float32)
    w_router = np.asarray(w_router, dtype=np.float32)
    b_router = np.asarray(b_router, dtype=np.float32)
    w1 = np.asarray(w1, dtype=np.float32)
    b1 = np.asarray(b1, dtype=np.float32)
    w2 = np.asarray(w2, dtype=np.float32)
    b2 = np.asarray(b2, dtype=np.float32)

    toks_s, wts_s, offs, slot_expert, caps = _route(x, w_router, b_router)
    fs = tuple(_fsplit(c) for c in caps)
    gs = tuple(c - f for c, f in zip(caps, fs))
    CT = sum(caps)
    FT = sum(fs)
    GT = sum(gs)
    soff = [0, caps[0]]
    foff = [0, fs[0]]
    goff = [0, gs[0]]

    nc = _build_program((caps, fs))

    xT = np.ascontiguousarray(x.T)          # [D, B] f32
    x8full = q8(xT * np.float32(SX))        # [D, B] fp8
    xbfull = xT.astype(BF16)                # [D, B] bf16

    in_maps = []
    tok_lists = []
    for c in range(N_CORES):
        x8core = np.zeros((D, FT), dtype=E4)
        xbcore = np.zeros((D, GT), dtype=BF16)
        core_toks = []
        for s in range(S):
            e = slot_expert[c][s]
            te = toks_s[offs[e] : offs[e + 1]]
            we = wts_s[offs[e] : offs[e + 1]]
            o = np.argsort(we, kind="stable")  # ascending combine weight
            te, we = te[o], we[o]
            core_toks.append((te, we, e))
            F = min(fs[s], len(te))
            x8core[:, foff[s] : foff[s] + F] = x8full[:, te[:F]]
            xbcore[:, goff[s] : goff[s] + len(te) - F] = xbfull[:, te[F:]]
        tok_lists.append(core_toks)
        x8core = np.ascontiguousarray(
            x8core.reshape(KQ1, 2, P, FT).transpose(2, 0, 1, 3)
        )
        xbcore = np.ascontiguousarray(
            xbcore.reshape(KO1, P, GT).transpose(1, 0, 2)
        )
        es = [slot_expert[c][s] for s in range(S)]

        w1q = q8(w1[es] * np.float32(SW1))  # [S, D, U] fp8
        w1fc = np.ascontiguousarray(
            w1q.reshape(S, KQ1, 2, P, M1, P).transpose(0, 4, 3, 1, 2, 5)
        )  # [S, M1, P, KQ1, 2, P]
        w1bc = np.ascontiguousarray(
            w1[es].astype(BF16).reshape(S, KO1, P, M1, P).transpose(0, 3, 2, 1, 4)
        )  # [S, M1, P, KO1, P]
        b1c = np.ascontiguousarray(b1[es].reshape(S, M1, P).transpose(0, 2, 1))
        w2q = q8(w2[es] * np.float32(SW2))  # [S, U, D] fp8
        w2fc = np.ascontiguousarray(
            w2q.reshape(S, KQ2, 2, P, M2, P).transpose(0, 4, 3, 1, 2, 5)
        )  # [S, M2, P, KQ2, 2, P]
        in_maps.append(
            {"xT8": x8core, "xTb": xbcore, "w1f": w1fc, "w1b": w1bc,
             "b1s": b1c, "w2f": w2fc}
        )

    kw = {}
    if TRACE:
        kw = dict(trace=True)
        if TRACE_CORES is not None:
            kw["trace_cores"] = TRACE_CORES
    res = run_bass_kernel_spmd(nc, in_maps, core_ids=list(range(N_CORES)), **kw)
    global LAST_RESULTS
    LAST_RESULTS = res

    out = x.copy()
    inv_sw2 = np.float32(1.0 / SW2)
    for c in range(N_CORES):
        yTc = np.asarray(res.results[c]["yT"]).astype(np.float32)  # [P, M2, CT]
        for s in range(S):
            te, we, e = tok_lists[c][s]
            n = len(te)
            if n == 0:
                continue
            y2 = yTc[:, :, soff[s] : soff[s] + n]
            y2 = y2.transpose(1, 0, 2).reshape(D, n)
            out[te] += we[:, None] * (y2.T * inv_sw2 + b2[e])
    return out


# revision 3
# speedup vs baseline: 1.0222x; 1.0222x over previous
"""MoE (top-2 routing, 16 experts, silu MLP) on 8 Trainium2 NeuronCores.

Mixed-precision expert parallelism:
  - Host: router (top-2 + softmax), dispatch. Each core owns 2 expert slots
    (largest-8 by token count paired with smallest-8). Within each expert's
    batch, tokens are sorted by ascending combine weight; the first F_s
    (~70% of capacity) low-weight tokens take the "fast" path (layer 1 in
    fp8-e4m3 DoubleRow matmuls), the high-weight rest take the "safe" path
    (layer 1 in bf16). Layer 2 is fp8 for all tokens. Error adds in
    quadrature over tokens, so putting fp8 on the low-weight pairs keeps the
    global rel-err ~1.9e-2 while most FLOPs run at the 2x fp8 rate.
  - Device per slot: h = silu(z) written fp8 straight into SBUF (no DRAM
    round trip; Act engine casts with exact RNE), then y*SW2 = h8 @ w2h
    (fp8 DoubleRow), y streamed out bf16.
  - Host: out = x + sum_e cw_e * (y_e/SW2 + b2_e) scatter-added.

fp8 scales (powers of 2): x*16, w1*512 (silu dequant via act scale 1/8192),
w2*256 (dequant folded into host combine).
"""

import sys
import types

import ml_dtypes
import numpy as np

B, D, E, U, TOPK = 16384, 1024, 16, 4096, 2
N_CORES = 8
S = E // N_CORES
P = 128
KQ1 = D // 256    # 4   fp8 double-k tiles, layer 1
KO1 = D // P      # 8   bf16 k tiles, layer 1
M1 = U // P       # 32
KQ2 = U // 256    # 16  fp8 double-k tiles, layer 2
M2 = D // P       # 8
FFRAC = 0.81      # fast-path fraction of each slot capacity

NCHF = 512        # fast-path chunk width (DoubleRow N)
NCHS = 512        # safe-path chunk width
NCH2 = 512        # layer-2 chunk width
SLAB = 512        # x DMA slab width

SX = 16.0
SW1 = 512.0
SW2 = 256.0
SCL1 = 1.0 / (SX * SW1)

E4 = ml_dtypes.float8_e4m3
BF16 = ml_dtypes.bfloat16

LAST_RESULTS = None
TRACE = False
TRACE_CORES = None


def _install_ntff_hook_shim():
    if "antenv.axon_hooks" in sys.modules:
        return
    try:
        import antenv.axon_hooks  # noqa: F401

        return
    except ImportError:
        pass
    try:
        import antenv
    except ImportError:
        return
    mod = types.ModuleType("antenv.axon_hooks")
    mod._hook = None

    def set_axon_ntff_profile_hook(h):
        mod._hook = h

    def get_axon_ntff_profile_hook():
        return mod._hook

    mod.set_axon_ntff_profile_hook = set_axon_ntff_profile_hook
    mod.get_axon_ntff_profile_hook = get_axon_ntff_profile_hook
    sys.modules["antenv.axon_hooks"] = mod
    antenv.axon_hooks = mod
    try:
        from trn_agent_boot.trn_boot import _ntff_profile_via_ctypes

        hook = _ntff_profile_via_ctypes("/opt/axon/libaxon_pjrt.so")
        if hook is not None:
            mod._hook = hook
    except Exception:
        pass


def _chunks(total, step):
    out = []
    c = 0
    while c < total:
        out.append((c, min(step, total - c)))
        c += step
    return out


def q8(v):
    return np.clip(v, -240.0, 240.0).astype(E4)


def _fsplit(cap):
    f = int(round(FFRAC * cap / 256.0)) * 256
    return min(max(f, 256), cap)


_PROGRAM_CACHE = {}


def _build_program(key):
    if key in _PROGRAM_CACHE:
        return _PROGRAM_CACHE[key]
    caps, fs = key

    import concourse.tile as tile
    from concourse import bacc, mybir

    f32 = mybir.dt.float32
    bf16 = mybir.dt.bfloat16
    f8 = mybir.dt.float8e4
    Silu = mybir.ActivationFunctionType.Silu
    DR = mybir.MatmulPerfMode.DoubleRow

    gs = tuple(c - f for c, f in zip(caps, fs))
    CT = sum(caps)
    FT = sum(fs)
    GT = sum(gs)
    CAP = max(caps)
    soff = [0, caps[0]]
    foff = [0, fs[0]]
    goff = [0, gs[0]]

    nc = bacc.Bacc(None, target_bir_lowering=False, debug=False)
    # fast x: [p, kq, i, n], input dim d = kq*256 + i*128 + p
    xT8 = nc.dram_tensor("xT8", [P, KQ1, 2, FT], f8, kind="ExternalInput")
    # safe x: [p, k, n], d = k*128 + p
    xTb = nc.dram_tensor("xTb", [P, KO1, GT], bf16, kind="ExternalInput")
    w1f = nc.dram_tensor("w1f", [S, M1, P, KQ1, 2, P], f8, kind="ExternalInput")
    w1b = nc.dram_tensor("w1b", [S, M1, P, KO1, P], bf16, kind="ExternalInput")
    b1s = nc.dram_tensor("b1s", [S, P, M1], f32, kind="ExternalInput")
    w2f = nc.dram_tensor("w2f", [S, M2, P, KQ2, 2, P], f8, kind="ExternalInput")
    # y*SW2: [p, m2, n], out dim d = m2*128 + p; cols = [fast | safe] per slot
    yT = nc.dram_tensor("yT", [P, M2, CT], bf16, kind="ExternalOutput")

    n_f_slabs = [len(_chunks(f, SLAB)) for f in fs]
    n_s_slabs = [len(_chunks(g, SLAB)) for g in gs]

    with tile.TileContext(nc) as tc:
        with (
            tc.tile_pool(name="bias", bufs=1) as biasp,
            tc.tile_pool(name="hbuf", bufs=1) as hpool,
            tc.tile_pool(name="x8sb", bufs=sum(n_f_slabs) + 1) as x8pool,
            tc.tile_pool(name="xbsb", bufs=sum(n_s_slabs) + 1) as xbpool,
            tc.tile_pool(name="w1fp", bufs=3) as w1fpool,
            tc.tile_pool(name="w1bp", bufs=3) as w1bpool,
            tc.tile_pool(name="w2fp", bufs=3) as w2fpool,
            tc.tile_pool(name="psum", bufs=6, space="PSUM") as psump,
            tc.tile_pool(name="yt", bufs=6) as ypool,
        ):
            b1_sb = biasp.tile([P, S, M1], f32, tag="b1")
            for s in range(S):
                nc.sync.dma_start(b1_sb[:, s, :], b1s[s])

            # PE clock warmup: the engine idles ~10us at startup waiting for
            # x/w DMA, and the first ~3us of execution run at reduced pstate.
            # Burn that window on throwaway matmuls (inputs: the tiny b1 tile)
            # so real matmuls start at full clock.
            warm = psump.tile([P, 16], f32, tag="ps", name="warm")
            for r in range(24):
                nc.tensor.matmul(
                    warm[:16, :16], b1_sb[:, 0, 0:16], b1_sb[:, 0, 0:16],
                    start=(r == 0), stop=(r == 23),
                )

            # x slabs; slot-0 immediately (fast/safe interleaved so the first
            # safe chunk isn't starved), slot-1 time-gated so the bulk
            # prefetch doesn't starve the w1 stream during slot-0 l1.
            t_l1 = 30_000 + int(caps[0] * 32 * 2.6)
            x8cs = [[], []]
            xbcs = [[], []]

            # First-m weight tiles, hoisted so their DMAs lead the gpsimd and
            # scalar queues at t=0.
            wtf0 = w1fpool.tile([P, KQ1, 2, P], f8, tag="w1f", name="w1f_pre")
            nc.gpsimd.dma_start(wtf0[:], w1f[0, 0])
            wtb0 = w1bpool.tile([P, KO1, P], bf16, tag="w1b", name="w1b_pre")
            nc.scalar.dma_start(wtb0[:], w1b[0, 0])

            def load_x8(s, si, c0, w, engs):
                xc = x8pool.tile([P, KQ1, 2, SLAB], f8, tag="x8", name=f"x8_{s}_{si}")
                for kq in range(KQ1):
                    for i in range(2):
                        engs[(kq * 2 + i) * len(engs) // 8].dma_start(
                            xc[:, kq, i, :w],
                            xT8[:, kq, i, foff[s] + c0 : foff[s] + c0 + w],
                        )
                x8cs[s].append(xc)

            def load_xb(s, si, c0, w, engs):
                xc = xbpool.tile([P, KO1, SLAB], bf16, tag="xb", name=f"xb_{s}_{si}")
                for k in range(KO1):
                    engs[k * len(engs) // 8].dma_start(
                        xc[:, k, :w],
                        xTb[:, k, goff[s] + c0 : goff[s] + c0 + w],
                    )
                xbcs[s].append(xc)

            fch = {s: _chunks(fs[s], SLAB) for s in range(S)}
            gch = {s: _chunks(gs[s], SLAB) for s in range(S)}
            # slot-0 startup: spread slabs over the three DMA queues so the
            # first m-sweep isn't serialized behind one ~90 GB/s queue.
            # slab 0 split across all three queues (shortest time-to-first-
            # matmul); later slabs each on one queue.
            x8q = [[nc.sync, nc.scalar, nc.gpsimd], [nc.scalar], [nc.gpsimd],
                   [nc.sync], [nc.scalar]]
            for si, (c0, w) in enumerate(fch[0]):
                load_x8(0, si, c0, w, x8q[si % len(x8q)])
            for si, (c0, w) in enumerate(gch[0]):
                load_xb(0, si, c0, w, [nc.sync, nc.scalar] if si == 0 else [nc.sync])
            for si, (c0, w) in enumerate(fch[1]):
                with tc.tile_wait_until((0.20 + 0.07 * si) * t_l1 / 1e6):
                    load_x8(1, si, c0, w, [nc.sync])
            for si, (c0, w) in enumerate(gch[1]):
                with tc.tile_wait_until((0.55 + 0.10 * si) * t_l1 / 1e6):
                    load_xb(1, si, c0, w, [nc.sync])

            for s in range(S):
                # ---- layer 1 ----
                h_sb = hpool.tile([P, M1, CAP], f8, tag="h", name=f"h{s}")
                for m in range(M1):
                    if s == 0 and m == 0:
                        wtf, wtb = wtf0, wtb0
                    else:
                        wtf = w1fpool.tile([P, KQ1, 2, P], f8, tag="w1f", name=f"w1f_{s}_{m}")
                        nc.gpsimd.dma_start(wtf[:], w1f[s, m])
                        wtb = w1bpool.tile([P, KO1, P], bf16, tag="w1b", name=f"w1b_{s}_{m}")
                        # first few on the (empty at startup) Act queue; the
                        # rest on sync behind the x-slab stream.
                        (nc.scalar if s == 0 and m < 3 else nc.sync).dma_start(
                            wtb[:], w1b[s, m]
                        )
                    # fast fp8 chunks -> h[:, m, 0:F]; for the very first m,
                    # visit chunks in DMA-arrival order (slab 0 split/early,
                    # slab 2 on gpsimd lands before slab 1 on scalar).
                    fastch = _chunks(fs[s], NCHF)
                    if s == 0 and m == 0 and len(fastch) == 3:
                        fastch = [fastch[0], fastch[2], fastch[1]]
                    for c0, w in fastch:
                        slab, off = divmod(c0, SLAB)
                        xc = x8cs[s][slab]
                        ps = psump.tile([P, NCHF], f32, tag="ps", name=f"psf_{s}_{m}_{c0}")
                        for kq in range(KQ1):
                            nc.tensor.matmul(
                                ps[:, :w],
                                wtf[:, kq],
                                xc[:, kq, :, off : off + w],
                                start=(kq == 0),
                                stop=(kq == KQ1 - 1),
                                perf_mode=DR,
                            )
                        nc.scalar.activation(
                            h_sb[:, m, c0 : c0 + w], ps[:, :w], Silu,
                            bias=b1_sb[:, s, m : m + 1], scale=SCL1,
                        )
                    # safe bf16 chunks -> h[:, m, F:C]
                    for c0, w in _chunks(gs[s], NCHS):
                        slab, off = divmod(c0, SLAB)
                        xc = xbcs[s][slab]
                        ps = psump.tile([P, NCHS], f32, tag="ps", name=f"pss_{s}_{m}_{c0}")
                        for k in range(KO1):
                            nc.tensor.matmul(
                                ps[:, :w],
                                wtb[:, k],
                                xc[:, k, off : off + w],
                                start=(k == 0),
                                stop=(k == KO1 - 1),
                            )
                        nc.scalar.activation(
                            h_sb[:, m, fs[s] + c0 : fs[s] + c0 + w], ps[:, :w], Silu,
                            bias=b1_sb[:, s, m : m + 1],
                        )

                # ---- layer 2: y*SW2 = h8 @ w2h (fp8 DoubleRow) ----
                for m2 in range(M2):
                    wt2 = w2fpool.tile([P, KQ2, 2, P], f8, tag="w2f", name=f"w2f_{s}_{m2}")
                    nc.scalar.dma_start(wt2[:], w2f[s, m2])
                    for c0, w in _chunks(caps[s], NCH2):
                        ps = psump.tile([P, NCH2], f32, tag="ps", name=f"ps2_{s}_{m2}_{c0}")
                        for kq in range(KQ2):
                            nc.tensor.matmul(
                                ps[:, :w],
                                wt2[:, kq],
                                h_sb[:, 2 * kq : 2 * kq + 2, c0 : c0 + w],
                                start=(kq == 0),
                                stop=(kq == KQ2 - 1),
                                perf_mode=DR,
                            )
                        yt = ypool.tile([P, NCH2], bf16, tag="yt", name=f"yt_{s}_{m2}_{c0}")
                        nc.vector.tensor_copy(yt[:, :w], ps[:, :w])
                        nc.gpsimd.dma_start(
                            yT[:, m2, soff[s] + c0 : soff[s] + c0 + w], yt[:, :w]
                        )

    nc.compile()
    _PROGRAM_CACHE[key] = nc
    return nc


def _route(x, w_router, b_router):
    logits = x @ w_router + b_router
    idx2 = np.argpartition(-logits, TOPK, axis=1)[:, :TOPK]
    vals = np.take_along_axis(logits, idx2, axis=1)
    order = np.argsort(-vals, axis=1)
    topk_i = np.take_along_axis(idx2, order, axis=1)
    topk_v = np.take_along_axis(vals, order, axis=1)
    topk_v = topk_v - topk_v.max(axis=1, keepdims=True)
    ew = np.exp(topk_v)
    cw = ew / ew.sum(axis=1, keepdims=True)

    eids = topk_i.ravel()
    toks = np.repeat(np.arange(B, dtype=np.int64), TOPK)
    wts = cw.ravel().astype(np.float32)
    perm = np.argsort(eids, kind="stable")
    toks_s, wts_s = toks[perm], wts[perm]
    counts = np.bincount(eids, minlength=E)
    offs = np.concatenate([[0], np.cumsum(counts)])

    rank = np.argsort(-counts, kind="stable")
    slot_expert = [[int(rank[s * N_CORES + c]) for s in range(S)] for c in range(N_CORES)]
    caps = tuple(
        max(512, int(max(counts[rank[s * N_CORES + c]] for c in range(N_CORES))))
        for s in range(S)
    )
    return toks_s, wts_s, offs, slot_expert, caps


def kernel(x, w_router, b_router, w1, b1, w2, b2):
    _install_ntff_hook_shim()
    from concourse.bass_utils import run_bass_kernel_spmd

    x = np.asarray(x, dtype=np.float32)
    w_router = np.asarray(w_router, dtype=np.float32)
    b_router = np.asarray(b_router, dtype=np.float32)
    w1 = np.asarray(w1, dtype=np.float32)
    b1 = np.asarray(b1, dtype=np.float32)
    w2 = np.asarray(w2, dtype=np.float32)
    b2 = np.asarray(b2, dtype=np.float32)

    toks_s, wts_s, offs, slot_expert, caps = _route(x, w_router, b_router)
    fs = tuple(_fsplit(c) for c in caps)
    gs = tuple(c - f for c, f in zip(caps, fs))
    CT = sum(caps)
    FT = sum(fs)
    GT = sum(gs)
    soff = [0, caps[0]]
    foff = [0, fs[0]]
    goff = [0, gs[0]]

    nc = _build_program((caps, fs))

    xT = np.ascontiguousarray(x.T)          # [D, B] f32
    x8full = q8(xT * np.float32(SX))        # [D, B] fp8
    xbfull = xT.astype(BF16)                # [D, B] bf16

    in_maps = []
    tok_lists = []
    for c in range(N_CORES):
        x8core = np.zeros((D, FT), dtype=E4)
        xbcore = np.zeros((D, GT), dtype=BF16)
        core_toks = []
        for s in range(S):
            e = slot_expert[c][s]
            te = toks_s[offs[e] : offs[e + 1]]
            we = wts_s[offs[e] : offs[e + 1]]
            o = np.argsort(we, kind="stable")  # ascending combine weight
            te, we = te[o], we[o]
            core_toks.append((te, we, e))
            F = min(fs[s], len(te))
            x8core[:, foff[s] : foff[s] + F] = x8full[:, te[:F]]
            xbcore[:, goff[s] : goff[s] + len(te) - F] = xbfull[:, te[F:]]
        tok_lists.append(core_toks)
        x8core = np.ascontiguousarray(
            x8core.reshape(KQ1, 2, P, FT).transpose(2, 0, 1, 3)
        )
        xbcore = np.ascontiguousarray(
            xbcore.reshape(KO1, P, GT).transpose(1, 0, 2)
        )
        es = [slot_expert[c][s] for s in range(S)]

        w1q = q8(w1[es] * np.float32(SW1))  # [S, D, U] fp8
        w1fc = np.ascontiguousarray(
            w1q.reshape(S, KQ1, 2, P, M1, P).transpose(0, 4, 3, 1, 2, 5)
        )  # [S, M1, P, KQ1, 2, P]
        w1bc = np.ascontiguousarray(
            w1[es].astype(BF16).reshape(S, KO1, P, M1, P).transpose(0, 3, 2, 1, 4)
        )  # [S, M1, P, KO1, P]
        b1c = np.ascontiguousarray(b1[es].reshape(S, M1, P).transpose(0, 2, 1))
        w2q = q8(w2[es] * np.float32(SW2))  # [S, U, D] fp8
        w2fc = np.ascontiguousarray(
            w2q.reshape(S, KQ2, 2, P, M2, P).transpose(0, 4, 3, 1, 2, 5)
        )  # [S, M2, P, KQ2, 2, P]
        in_maps.append(
            {"xT8": x8core, "xTb": xbcore, "w1f": w1fc, "w1b": w1bc,
             "b1s": b1c, "w2f": w2fc}
        )

    kw = {}
    if TRACE:
        kw = dict(trace=True)
        if TRACE_CORES is not None:
            kw["trace_cores"] = TRACE_CORES
    res = run_bass_kernel_spmd(nc, in_maps, core_ids=list(range(N_CORES)), **kw)
    global LAST_RESULTS
    LAST_RESULTS = res

    out = x.copy()
    inv_sw2 = np.float32(1.0 / SW2)
    for c in range(N_CORES):
        yTc = np.asarray(res.results[c]["yT"]).astype(np.float32)  # [P, M2, CT]
        for s in range(S):
            te, we, e = tok_lists[c][s]
            n = len(te)
            if n == 0:
                continue
            y2 = yTc[:, :, soff[s] : soff[s] + n]
            y2 = y2.transpose(1, 0, 2).reshape(D, n)
            out[te] += we[:, None] * (y2.T * inv_sw2 + b2[e])
    return out


# revision 4
# speedup vs baseline: 1.0225x; 1.0003x over previous
"""MoE (top-2 routing, 16 experts, silu MLP) on 8 Trainium2 NeuronCores.

Mixed-precision expert parallelism:
  - Host: router (top-2 + softmax), dispatch. Each core owns 2 expert slots
    (largest-8 by token count paired with smallest-8). Within each expert's
    batch, tokens are sorted by ascending combine weight; the first F_s
    (~70% of capacity) low-weight tokens take the "fast" path (layer 1 in
    fp8-e4m3 DoubleRow matmuls), the high-weight rest take the "safe" path
    (layer 1 in bf16). Layer 2 is fp8 for all tokens. Error adds in
    quadrature over tokens, so putting fp8 on the low-weight pairs keeps the
    global rel-err ~1.9e-2 while most FLOPs run at the 2x fp8 rate.
  - Device per slot: h = silu(z) written fp8 straight into SBUF (no DRAM
    round trip; Act engine casts with exact RNE), then y*SW2 = h8 @ w2h
    (fp8 DoubleRow), y streamed out bf16.
  - Host: out = x + sum_e cw_e * (y_e/SW2 + b2_e) scatter-added.

fp8 scales (powers of 2): x*16, w1*512 (silu dequant via act scale 1/8192),
w2*256 (dequant folded into host combine).
"""

import sys
import types

import ml_dtypes
import numpy as np

B, D, E, U, TOPK = 16384, 1024, 16, 4096, 2
N_CORES = 8
S = E // N_CORES
P = 128
KQ1 = D // 256    # 4   fp8 double-k tiles, layer 1
KO1 = D // P      # 8   bf16 k tiles, layer 1
M1 = U // P       # 32
KQ2 = U // 256    # 16  fp8 double-k tiles, layer 2
M2 = D // P       # 8
FFRAC = 0.81      # fast-path fraction of each slot capacity
FBUMP0 = 128      # extra fast columns on slot 0 (paid for by b2 calibration)

NCHF = 512        # fast-path chunk width (DoubleRow N)
NCHS = 512        # safe-path chunk width
NCH2 = 512        # layer-2 chunk width
SLAB = 512        # x DMA slab width

SX = 16.0
SW1 = 512.0
SW2 = 256.0
SCL1 = 1.0 / (SX * SW1)

E4 = ml_dtypes.float8_e4m3
BF16 = ml_dtypes.bfloat16

LAST_RESULTS = None
TRACE = False
TRACE_CORES = None


def _install_ntff_hook_shim():
    if "antenv.axon_hooks" in sys.modules:
        return
    try:
        import antenv.axon_hooks  # noqa: F401

        return
    except ImportError:
        pass
    try:
        import antenv
    except ImportError:
        return
    mod = types.ModuleType("antenv.axon_hooks")
    mod._hook = None

    def set_axon_ntff_profile_hook(h):
        mod._hook = h

    def get_axon_ntff_profile_hook():
        return mod._hook

    mod.set_axon_ntff_profile_hook = set_axon_ntff_profile_hook
    mod.get_axon_ntff_profile_hook = get_axon_ntff_profile_hook
    sys.modules["antenv.axon_hooks"] = mod
    antenv.axon_hooks = mod
    try:
        from trn_agent_boot.trn_boot import _ntff_profile_via_ctypes

        hook = _ntff_profile_via_ctypes("/opt/axon/libaxon_pjrt.so")
        if hook is not None:
            mod._hook = hook
    except Exception:
        pass


def _chunks(total, step):
    out = []
    c = 0
    while c < total:
        out.append((c, min(step, total - c)))
        c += step
    return out


def q8(v):
    return np.clip(v, -240.0, 240.0).astype(E4)


def _fsplit(cap):
    f = int(round(FFRAC * cap / 256.0)) * 256
    return min(max(f, 256), cap)


_PROGRAM_CACHE = {}


def _build_program(key):
    if key in _PROGRAM_CACHE:
        return _PROGRAM_CACHE[key]
    caps, fs = key

    import concourse.tile as tile
    from concourse import bacc, mybir

    f32 = mybir.dt.float32
    bf16 = mybir.dt.bfloat16
    f8 = mybir.dt.float8e4
    Silu = mybir.ActivationFunctionType.Silu
    DR = mybir.MatmulPerfMode.DoubleRow

    gs = tuple(c - f for c, f in zip(caps, fs))
    CT = sum(caps)
    FT = sum(fs)
    GT = sum(gs)
    CAP = max(caps)
    soff = [0, caps[0]]
    foff = [0, fs[0]]
    goff = [0, gs[0]]

    nc = bacc.Bacc(None, target_bir_lowering=False, debug=False)
    # fast x: [p, kq, i, n], input dim d = kq*256 + i*128 + p
    xT8 = nc.dram_tensor("xT8", [P, KQ1, 2, FT], f8, kind="ExternalInput")
    # safe x: [p, k, n], d = k*128 + p
    xTb = nc.dram_tensor("xTb", [P, KO1, GT], bf16, kind="ExternalInput")
    w1f = nc.dram_tensor("w1f", [S, M1, P, KQ1, 2, P], f8, kind="ExternalInput")
    w1b = nc.dram_tensor("w1b", [S, M1, P, KO1, P], bf16, kind="ExternalInput")
    b1s = nc.dram_tensor("b1s", [S, P, M1], f32, kind="ExternalInput")
    w2f = nc.dram_tensor("w2f", [S, M2, P, KQ2, 2, P], f8, kind="ExternalInput")
    # y*SW2: [p, m2, n], out dim d = m2*128 + p; cols = [fast | safe] per slot
    yT = nc.dram_tensor("yT", [P, M2, CT], bf16, kind="ExternalOutput")

    n_f_slabs = [len(_chunks(f, SLAB)) for f in fs]
    n_s_slabs = [len(_chunks(g, SLAB)) for g in gs]

    with tile.TileContext(nc) as tc:
        with (
            tc.tile_pool(name="bias", bufs=1) as biasp,
            tc.tile_pool(name="hbuf", bufs=1) as hpool,
            tc.tile_pool(name="x8sb", bufs=sum(n_f_slabs) + 1) as x8pool,
            tc.tile_pool(name="xbsb", bufs=sum(n_s_slabs) + 1) as xbpool,
            tc.tile_pool(name="w1fp", bufs=3) as w1fpool,
            tc.tile_pool(name="w1bp", bufs=3) as w1bpool,
            tc.tile_pool(name="w2fp", bufs=3) as w2fpool,
            tc.tile_pool(name="psum", bufs=6, space="PSUM") as psump,
            tc.tile_pool(name="yt", bufs=6) as ypool,
        ):
            b1_sb = biasp.tile([P, S, M1], f32, tag="b1")
            for s in range(S):
                nc.sync.dma_start(b1_sb[:, s, :], b1s[s])

            # PE clock warmup: the engine idles ~10us at startup waiting for
            # x/w DMA, and the first ~3us of execution run at reduced pstate.
            # Burn that window on throwaway matmuls (inputs: the tiny b1 tile)
            # so real matmuls start at full clock.
            warm = psump.tile([P, 16], f32, tag="ps", name="warm")
            for r in range(24):
                nc.tensor.matmul(
                    warm[:16, :16], b1_sb[:, 0, 0:16], b1_sb[:, 0, 0:16],
                    start=(r == 0), stop=(r == 23),
                )

            # x slabs; slot-0 immediately (fast/safe interleaved so the first
            # safe chunk isn't starved), slot-1 time-gated so the bulk
            # prefetch doesn't starve the w1 stream during slot-0 l1.
            t_l1 = 30_000 + int(caps[0] * 32 * 2.6)
            x8cs = [[], []]
            xbcs = [[], []]

            # First-m weight tiles, hoisted so their DMAs lead the gpsimd and
            # scalar queues at t=0.
            wtf0 = w1fpool.tile([P, KQ1, 2, P], f8, tag="w1f", name="w1f_pre")
            nc.gpsimd.dma_start(wtf0[:], w1f[0, 0])
            wtb0 = w1bpool.tile([P, KO1, P], bf16, tag="w1b", name="w1b_pre")
            nc.scalar.dma_start(wtb0[:], w1b[0, 0])

            def load_x8(s, si, c0, w, engs):
                xc = x8pool.tile([P, KQ1, 2, SLAB], f8, tag="x8", name=f"x8_{s}_{si}")
                for kq in range(KQ1):
                    for i in range(2):
                        engs[(kq * 2 + i) * len(engs) // 8].dma_start(
                            xc[:, kq, i, :w],
                            xT8[:, kq, i, foff[s] + c0 : foff[s] + c0 + w],
                        )
                x8cs[s].append(xc)

            def load_xb(s, si, c0, w, engs):
                xc = xbpool.tile([P, KO1, SLAB], bf16, tag="xb", name=f"xb_{s}_{si}")
                for k in range(KO1):
                    engs[k * len(engs) // 8].dma_start(
                        xc[:, k, :w],
                        xTb[:, k, goff[s] + c0 : goff[s] + c0 + w],
                    )
                xbcs[s].append(xc)

            fch = {s: _chunks(fs[s], SLAB) for s in range(S)}
            gch = {s: _chunks(gs[s], SLAB) for s in range(S)}
            # slot-0 startup: spread slabs over the three DMA queues so the
            # first m-sweep isn't serialized behind one ~90 GB/s queue.
            # slab 0 split across all three queues (shortest time-to-first-
            # matmul); later slabs each on one queue.
            x8q = [[nc.sync, nc.scalar, nc.gpsimd], [nc.scalar], [nc.gpsimd],
                   [nc.sync], [nc.scalar]]
            for si, (c0, w) in enumerate(fch[0]):
                load_x8(0, si, c0, w, x8q[si % len(x8q)])
            for si, (c0, w) in enumerate(gch[0]):
                load_xb(0, si, c0, w, [nc.sync, nc.scalar] if si == 0 else [nc.sync])
            for si, (c0, w) in enumerate(fch[1]):
                with tc.tile_wait_until((0.20 + 0.07 * si) * t_l1 / 1e6):
                    load_x8(1, si, c0, w, [nc.sync])
            for si, (c0, w) in enumerate(gch[1]):
                with tc.tile_wait_until((0.55 + 0.10 * si) * t_l1 / 1e6):
                    load_xb(1, si, c0, w, [nc.sync])

            for s in range(S):
                # ---- layer 1 ----
                h_sb = hpool.tile([P, M1, CAP], f8, tag="h", name=f"h{s}")
                for m in range(M1):
                    if s == 0 and m == 0:
                        wtf, wtb = wtf0, wtb0
                    else:
                        wtf = w1fpool.tile([P, KQ1, 2, P], f8, tag="w1f", name=f"w1f_{s}_{m}")
                        nc.gpsimd.dma_start(wtf[:], w1f[s, m])
                        wtb = w1bpool.tile([P, KO1, P], bf16, tag="w1b", name=f"w1b_{s}_{m}")
                        # first few on the (empty at startup) Act queue; the
                        # rest on sync behind the x-slab stream.
                        (nc.scalar if s == 0 and m < 3 else nc.sync).dma_start(
                            wtb[:], w1b[s, m]
                        )
                    # fast fp8 chunks -> h[:, m, 0:F]; for the very first m,
                    # visit chunks in DMA-arrival order (slab 0 split/early,
                    # slab 2 on gpsimd lands before slab 1 on scalar).
                    fastch = _chunks(fs[s], NCHF)
                    if s == 0 and m == 0 and len(fastch) == 3:
                        fastch = [fastch[0], fastch[2], fastch[1]]
                    for c0, w in fastch:
                        slab, off = divmod(c0, SLAB)
                        xc = x8cs[s][slab]
                        ps = psump.tile([P, NCHF], f32, tag="ps", name=f"psf_{s}_{m}_{c0}")
                        for kq in range(KQ1):
                            nc.tensor.matmul(
                                ps[:, :w],
                                wtf[:, kq],
                                xc[:, kq, :, off : off + w],
                                start=(kq == 0),
                                stop=(kq == KQ1 - 1),
                                perf_mode=DR,
                            )
                        nc.scalar.activation(
                            h_sb[:, m, c0 : c0 + w], ps[:, :w], Silu,
                            bias=b1_sb[:, s, m : m + 1], scale=SCL1,
                        )
                    # safe bf16 chunks -> h[:, m, F:C]
                    for c0, w in _chunks(gs[s], NCHS):
                        slab, off = divmod(c0, SLAB)
                        xc = xbcs[s][slab]
                        ps = psump.tile([P, NCHS], f32, tag="ps", name=f"pss_{s}_{m}_{c0}")
                        for k in range(KO1):
                            nc.tensor.matmul(
                                ps[:, :w],
                                wtb[:, k],
                                xc[:, k, off : off + w],
                                start=(k == 0),
                                stop=(k == KO1 - 1),
                            )
                        nc.scalar.activation(
                            h_sb[:, m, fs[s] + c0 : fs[s] + c0 + w], ps[:, :w], Silu,
                            bias=b1_sb[:, s, m : m + 1],
                        )

                # ---- layer 2: y*SW2 = h8 @ w2h (fp8 DoubleRow) ----
                for m2 in range(M2):
                    wt2 = w2fpool.tile([P, KQ2, 2, P], f8, tag="w2f", name=f"w2f_{s}_{m2}")
                    nc.scalar.dma_start(wt2[:], w2f[s, m2])
                    for c0, w in _chunks(caps[s], NCH2):
                        ps = psump.tile([P, NCH2], f32, tag="ps", name=f"ps2_{s}_{m2}_{c0}")
                        for kq in range(KQ2):
                            nc.tensor.matmul(
                                ps[:, :w],
                                wt2[:, kq],
                                h_sb[:, 2 * kq : 2 * kq + 2, c0 : c0 + w],
                                start=(kq == 0),
                                stop=(kq == KQ2 - 1),
                                perf_mode=DR,
                            )
                        yt = ypool.tile([P, NCH2], bf16, tag="yt", name=f"yt_{s}_{m2}_{c0}")
                        nc.vector.tensor_copy(yt[:, :w], ps[:, :w])
                        nc.gpsimd.dma_start(
                            yT[:, m2, soff[s] + c0 : soff[s] + c0 + w], yt[:, :w]
                        )

    nc.compile()
    _PROGRAM_CACHE[key] = nc
    return nc


def _route(x, w_router, b_router):
    logits = x @ w_router + b_router
    idx2 = np.argpartition(-logits, TOPK, axis=1)[:, :TOPK]
    vals = np.take_along_axis(logits, idx2, axis=1)
    order = np.argsort(-vals, axis=1)
    topk_i = np.take_along_axis(idx2, order, axis=1)
    topk_v = np.take_along_axis(vals, order, axis=1)
    topk_v = topk_v - topk_v.max(axis=1, keepdims=True)
    ew = np.exp(topk_v)
    cw = ew / ew.sum(axis=1, keepdims=True)

    eids = topk_i.ravel()
    toks = np.repeat(np.arange(B, dtype=np.int64), TOPK)
    wts = cw.ravel().astype(np.float32)
    perm = np.argsort(eids, kind="stable")
    toks_s, wts_s = toks[perm], wts[perm]
    counts = np.bincount(eids, minlength=E)
    offs = np.concatenate([[0], np.cumsum(counts)])

    rank = np.argsort(-counts, kind="stable")
    slot_expert = [[int(rank[s * N_CORES + c]) for s in range(S)] for c in range(N_CORES)]
    caps = tuple(
        max(512, int(max(counts[rank[s * N_CORES + c]] for c in range(N_CORES))))
        for s in range(S)
    )
    return toks_s, wts_s, offs, slot_expert, caps


def kernel(x, w_router, b_router, w1, b1, w2, b2):
    _install_ntff_hook_shim()
    from concourse.bass_utils import run_bass_kernel_spmd

    x = np.asarray(x, dtype=np.float32)
    w_router = np.asarray(w_router, dtype=np.float32)
    b_router = np.asarray(b_router, dtype=np.float32)
    w1 = np.asarray(w1, dtype=np.float32)
    b1 = np.asarray(b1, dtype=np.float32)
    w2 = np.asarray(w2, dtype=np.float32)
    b2 = np.asarray(b2, dtype=np.float32)

    toks_s, wts_s, offs, slot_expert, caps = _route(x, w_router, b_router)
    fs = tuple(_fsplit(c) for c in caps)
    fs = (min(fs[0] + FBUMP0, caps[0]), fs[1])
    gs = tuple(c - f for c, f in zip(caps, fs))
    CT = sum(caps)
    FT = sum(fs)
    GT = sum(gs)
    soff = [0, caps[0]]
    foff = [0, fs[0]]
    goff = [0, gs[0]]

    nc = _build_program((caps, fs))

    xT = np.ascontiguousarray(x.T)          # [D, B] f32
    x8full = q8(xT * np.float32(SX))        # [D, B] fp8
    xbfull = xT.astype(BF16)                # [D, B] bf16

    in_maps = []
    tok_lists = []
    for c in range(N_CORES):
        x8core = np.zeros((D, FT), dtype=E4)
        xbcore = np.zeros((D, GT), dtype=BF16)
        core_toks = []
        for s in range(S):
            e = slot_expert[c][s]
            te = toks_s[offs[e] : offs[e + 1]]
            we = wts_s[offs[e] : offs[e + 1]]
            o = np.argsort(we, kind="stable")  # ascending combine weight
            te, we = te[o], we[o]
            core_toks.append((te, we, e))
            F = min(fs[s], len(te))
            x8core[:, foff[s] : foff[s] + F] = x8full[:, te[:F]]
            xbcore[:, goff[s] : goff[s] + len(te) - F] = xbfull[:, te[F:]]
        tok_lists.append(core_toks)
        x8core = np.ascontiguousarray(
            x8core.reshape(KQ1, 2, P, FT).transpose(2, 0, 1, 3)
        )
        xbcore = np.ascontiguousarray(
            xbcore.reshape(KO1, P, GT).transpose(1, 0, 2)
        )
        es = [slot_expert[c][s] for s in range(S)]

        w1q = q8(w1[es] * np.float32(SW1))  # [S, D, U] fp8
        w1fc = np.ascontiguousarray(
            w1q.reshape(S, KQ1, 2, P, M1, P).transpose(0, 4, 3, 1, 2, 5)
        )  # [S, M1, P, KQ1, 2, P]
        w1bc = np.ascontiguousarray(
            w1[es].astype(BF16).reshape(S, KO1, P, M1, P).transpose(0, 3, 2, 1, 4)
        )  # [S, M1, P, KO1, P]
        b1c = np.ascontiguousarray(b1[es].reshape(S, M1, P).transpose(0, 2, 1))
        w2q = q8(w2[es] * np.float32(SW2))  # [S, U, D] fp8
        w2fc = np.ascontiguousarray(
            w2q.reshape(S, KQ2, 2, P, M2, P).transpose(0, 4, 3, 1, 2, 5)
        )  # [S, M2, P, KQ2, 2, P]
        in_maps.append(
            {"xT8": x8core, "xTb": xbcore, "w1f": w1fc, "w1b": w1bc,
             "b1s": b1c, "w2f": w2fc}
        )

    kw = {}
    if TRACE:
        kw = dict(trace=True)
        if TRACE_CORES is not None:
            kw["trace_cores"] = TRACE_CORES
    res = run_bass_kernel_spmd(nc, in_maps, core_ids=list(range(N_CORES)), **kw)
    global LAST_RESULTS
    LAST_RESULTS = res

    # Calibrate b2: the w2 quantization error dW2 = w2*SW2 - Q(w2*SW2) is a
    # fixed matrix per expert, so the mean component of its per-token output
    # error, hbar @ dW2 (hbar = E[silu(z)] with z_u ~ N(b1_u, ||w1[:,u]||^2),
    # via Gauss-Hermite), is a constant vector correctable through b2.
    gh_t, gh_w = np.polynomial.hermite_e.hermegauss(64)
    gh_w = (gh_w / gh_w.sum()).astype(np.float32)
    b2_adj = np.empty_like(b2)
    for e in range(E):
        sig = np.linalg.norm(w1[e], axis=0)
        zz = b1[e][:, None] + sig[:, None] * gh_t[None, :].astype(np.float32)
        hbar = ((zz / (1.0 + np.exp(-zz))) * gh_w[None, :]).sum(axis=1)
        w2s = w2[e] * np.float32(SW2)
        dW2 = w2s - q8(w2s).astype(np.float32)
        b2_adj[e] = b2[e] + (hbar @ dW2) / np.float32(SW2)

    out = x.copy()
    inv_sw2 = np.float32(1.0 / SW2)
    for c in range(N_CORES):
        yTc = np.asarray(res.results[c]["yT"]).astype(np.float32)  # [P, M2, CT]
        for s in range(S):
            te, we, e = tok_lists[c][s]
            n = len(te)
            if n == 0:
                continue
            y2 = yTc[:, :, soff[s] : soff[s] + n]
            y2 = y2.transpose(1, 0, 2).reshape(D, n)
            out[te] += we[:, None] * (y2.T * inv_sw2 + b2_adj[e])
    return out
